# revision 72
# baseline (speedup 1.0000x reference)
"""Trainium2 Bass kernel: 1-layer transformer block w/ ALiBi bidirectional attention.

Sharding: data-parallel over batch (B=8) across 8 NeuronCores; zero collectives.

v3 (banded attention + pipeline rework), on top of v2's bf16 scheme:
  - Banded attention: ALiBi slopes decay so fast that blocks beyond
    B={2,2,3,5} (per slope) 128-blocks from the diagonal underflow to
    exactly 0 after exp (dropped mass < e^-21); scores/exp/pv do ~56%
    of the full masked work.  Validated ≤2e-12 vs full softmax.
  - q/k projections paired: full 128-wide stationary (2 heads per
    matmul) halves qk PE time; psum evacuated by ONE full-width DVE
    copy into a scratch tile, then two SBUF->SBUF DMAs partition-shift
    the per-head [64,S] halves into the augmented qTa/kTa tiles.
  - ALiBi per-t term rides the score matmul as a hi/lo pair of bf16
    aug rows (exact to 2^-18), so exp needs NO per-chunk bias and one
    ACT exp op spans a [128,1024] 2-bank psum tile: far fewer ops on
    the 352-cycle-overhead ACT engine.
  - LN rsqrt = exp(-0.5*ln(var+eps)): both Ln and Exp live in the
    natural_log_exp_and_others ACT table set (the insertion pass is
    fed a filtered table list so it picks that set), so the only table
    swaps are ln_exp -> gelu -> ln_exp (3 loads vs 9).
  - FFN2 runs per-s-chunk after each half's FFN1 (gt staged in SBUF),
    needing 1 psum accumulator instead of 4; psum plan is exactly 8
    banks: 2x[128,512] + 2x[128,1024] + 2x aux.
  - All-zero bias tensors (this model's setup) detected at runtime ->
    bias adds drop to plain copies/TTs.
  - HAM warm-up uses real matmuls (transposes don't count as PE-busy).
"""

import sys
import types

import ml_dtypes
import numpy as np

sys.path.insert(0, "/opt/trn_rl_repo")

import concourse.bass as bass  # noqa: E402
from concourse import bacc  # noqa: E402
import concourse.tile as tile  # noqa: E402
from concourse import mybir  # noqa: E402
from concourse.bass_utils import run_bass_kernel_spmd  # noqa: E402
import bass_rust as _bass_rust  # noqa: E402
from concourse.hw_specs import get_activation_tables  # noqa: E402

F32 = mybir.dt.float32
BF = mybir.dt.bfloat16
AF = mybir.ActivationFunctionType
OP = mybir.AluOpType

P = 128
B = 8
S = 1024
D = 512
H = 8
HD = 64
FFN = 4 * D
SM = S // P  # 8 sequence chunks
DK = D // P  # 4 feature chunks
FK = FFN // P  # 16 ffn chunks
EPS = 1e-5
N_CORES = 8
AUG = 3  # aug rows: (qrow|ones), (ones|krow_hi), (ones|krow_lo)

BF_NP = ml_dtypes.bfloat16

BANDS = [2, 2, 3, 5]  # kept 128-blocks (incl. diagonal) per slope index


def _slopes():
    half = H // 2
    base = 24.0 ** (1.0 / half)
    return (1.0 / base ** np.arange(1, half + 1)).astype(np.float64)


def _fwd(h):
    return h < H // 2


def _band(h):
    return BANDS[h % 4]


def _group(h, j):
    # s-range of score block-column group for t-chunk j (dense band)
    Bh = _band(h)
    if _fwd(h):  # keep t <= s, s - t < Bh*128
        s0 = j * P
        s1 = min(S, (j + Bh) * P)
    else:  # keep t >= s, t - s < Bh*128
        s0 = max(0, (j - Bh + 1) * P)
        s1 = (j + 1) * P
    return s0, s1 - s0


def _eoffs(h):
    offs, off = [], 0
    for j in range(SM):
        offs.append(off)
        off += _group(h, j)[1]
    return offs, off


def _score_tiles(h):
    """Chunk head h's dense score layout into psum tiles of <=1024 cols.

    Returns [(tile_off, tile_w, [(j, qTa_src_col, dst_col_in_tile, w)...])].
    Matmul pieces never cross an absolute 512 (bank) boundary.
    """
    offs, ew = _eoffs(h)
    pieces = []
    for j in range(SM):
        s0, w = _group(h, j)
        off = offs[j]
        pos = 0
        while pos < w:
            lim = 512 - ((off + pos) % 512)
            pw = min(w - pos, lim)
            pieces.append((j, s0 + pos, off + pos, pw))
            pos += pw
    tiles = []
    for t0 in range(0, ew, 1024):
        tw = min(1024, ew - t0)
        tp = [(j, ss, do - t0, w) for (j, ss, do, w) in pieces if t0 <= do < t0 + tw]
        tiles.append((t0, tw, tp))
    return tiles


def _diag_off(h, j):
    offs, _ = _eoffs(h)
    s0, _w = _group(h, j)
    return offs[j] + (j * P - s0)


def _pv_js(h, m):
    Bh = _band(h)
    if _fwd(h):
        return list(range(max(0, m - Bh + 1), m + 1))
    return list(range(m, min(SM, m + Bh)))


def _pv_col(h, j, m):
    offs, _ = _eoffs(h)
    s0, _w = _group(h, j)
    return offs[j] + (m * P - s0)


def _ew(h):
    return _eoffs(h)[1]


EW_S = _ew(0)  # 1920 (band 2 heads)
EW_L = _ew(3)  # 3840 (band 5; band-3 heads fit too)


def _patched_insert_act_table_loads(self):
    """Feed the table-load pass a filtered set list so Exp and Ln both
    resolve to natural_log_exp_and_others (greedy first-match would
    otherwise alternate exp_and_others / natural_log per call)."""
    has_activation = any(
        isinstance(i, mybir.InstActivation)
        for b in self.main_func.blocks
        for i in b.instructions
    )
    if not has_activation:
        return
    tables = []
    for name, fns in get_activation_tables(self.m.arch).items():
        fns = set(fns)
        if name in ("exp_and_others", "exp_and_friends"):
            fns.discard(AF.Exp)
        if name == "natural_log":
            fns.discard(AF.Ln)
        tables.append((name, fns))
    _bass_rust.insert_act_table_loads(self, tables)


def build_nc(gelu_mode="gelu", zero_bias=True):
    nc = bacc.Bacc("TRN2", target_bir_lowering=False, debug=False)
    nc.insert_act_table_loads = types.MethodType(_patched_insert_act_table_loads, nc)

    def din(name, shape, dt=F32):
        return nc.dram_tensor(name, list(shape), dt, kind="ExternalInput").ap()

    # weights arrive pre-rearranged to the on-chip [p, chunk, n] layout so
    # every load is a contiguous [128, N] DMA (cheap descriptor issue)
    d = {"zero_bias": zero_bias}
    d["xT0"] = din("xT0", (P, DK, 512), BF)  # s-halves as separate tensors:
    d["xT1"] = din("xT1", (P, DK, 512), BF)  # contiguous DMAs, cheap issue
    d["w_in"] = din("w_in", (P, DK, D), BF)
    d["wq"] = din("wq", (P, DK, D), BF)
    d["wk"] = din("wk", (P, DK, D), BF)
    d["wv"] = din("wv", (P, DK, D), BF)
    d["wo"] = din("wo", (P, DK, D), BF)
    d["w1"] = din("w1", (P, DK, FFN), BF)
    d["w2"] = din("w2", (P, FK, D), BF)
    d["w_out"] = din("w_out", (P, DK, D), BF)
    d["qaug"] = din("qaug", (3 * H, S), BF)  # per head: ones, ones, qrow
    d["kaug"] = din("kaug", (3 * H, S), BF)  # per head: krow_hi, krow_lo, ones
    d["maskf"] = din("maskf", (P, P), BF)
    d["maskb"] = din("maskb", (P, P), BF)
    d["ident"] = din("ident", (P, P), BF)
    if not zero_bias:
        d["b_in"] = din("b_in", (D,))
        d["bo"] = din("bo", (D,))
        d["b2"] = din("b2", (D,))
        d["b_out"] = din("b_out", (D,))
        d["bv"] = din("bv", (D,))
        d["bqp"] = din("bqp", (P, DK))  # paired q bias: [2-head dims, pair]
        d["b1c"] = din("b1c", (P, FK))
    d["out"] = nc.dram_tensor("out", [S, D], F32, kind="ExternalOutput").ap()

    with tile.TileContext(nc, pool_alloc_mode="queue") as tc:
        _emit(nc, tc, d, gelu_mode)
    nc.compile()
    return nc


def _emit(nc, tc, d, gelu_mode):
    pool = tc.alloc_tile_pool
    zb = d["zero_bias"]

    pc = pool(name="consts", bufs=1)
    pw = pool(name="weights", bufs=1)
    ph = pool(name="resid", bufs=2)  # h1/h2/h3 rotate, fp32
    phT = pool(name="transposed", bufs=2)  # hn1T/attnT2/hn2T/hn3T
    phn = pool(name="hn_nat", bufs=4)
    psm = pool(name="smalls", bufs=2)
    pva = pool(name="vaug", bufs=1)
    pattn = pool(name="attn_nat", bufs=1)
    posb = pool(name="outsb", bufs=2)
    # attention-only pools: allocated last (top of the pool stack) so they
    # can be released before the FFN gt pool is allocated.  expL needs 4
    # slots: the two big-band pairs run back-to-back at the pipeline head.
    pqk = pool(name="qkheads", bufs=4)
    pqp = pool(name="qkscratch", bufs=3)
    pexpS = pool(name="expTS", bufs=2)
    pexpL = pool(name="expTL", bufs=4)

    # psum: 2x mm (1 bank) + 2x sc (2 banks) + 2x aux (1 bank) = 8 banks
    pps = pool(name="ps", bufs=2, space="PSUM")

    def ps_mm(name, shape=None):
        return pps.tile(shape or [P, 512], F32, tag="mm", name=name)

    def ps_sc(name, shape=None):
        return pps.tile(shape or [P, 1024], F32, tag="sc", name=name)

    # ---- DMAs: ident first (warmup dep), then startup-critical tensors ----
    # warm-up source needs NO DMA (nothing arrives before ~12us): memset
    warmw = pc.tile([P, 512], BF, tag="warmw")
    nc.any.memset(warmw, 1.0)
    identB = pc.tile([P, P], BF, tag="ident")
    nc.sync.dma_start(out=identB, in_=d["ident"])
    win_sb = pw.tile([P, DK, D], BF, tag="w_in")
    nc.sync.dma_start(out=win_sb, in_=d["w_in"])
    xT_sb = pw.tile([P, 2, DK, 512], BF, tag="xT")
    nc.sync.dma_start(out=xT_sb[:, 0], in_=d["xT0"])
    nc.sync.dma_start(out=xT_sb[:, 1], in_=d["xT1"])

    def wload(name, shape, eng=nc.sync):
        t = pw.tile(shape, BF, tag=name)
        eng.dma_start(out=t, in_=d[name])
        return t

    # all weight loads on ONE queue in priority order: the 16 DMA engines
    # serve descriptors in enqueue order, so concurrent queues would let
    # late-needed weights cut ahead of the startup-critical xT/w_in
    wq_sb = wload("wq", [P, DK, D])
    wk_sb = wload("wk", [P, DK, D])
    wv_sb = wload("wv", [P, DK, D])

    maskf = pc.tile([P, P], BF, tag="maskf")
    nc.gpsimd.dma_start(out=maskf, in_=d["maskf"])
    maskb = pc.tile([P, P], BF, tag="maskb")
    nc.gpsimd.dma_start(out=maskb, in_=d["maskb"])

    wo_sb = wload("wo", [P, DK, D])
    w1_sb = wload("w1", [P, DK, FFN])
    w2_sb = wload("w2", [P, FK, D])
    wout_sb = wload("w_out", [P, DK, D])

    epsc = pc.tile([P, 1], F32, tag="epsc")
    nc.any.memset(epsc, EPS)

    if not zb:
        def bcast(name, shape=None):
            t = pc.tile(shape or [P, D], F32, tag=name + "B")
            nc.gpsimd.dma_start(out=t, in_=d[name].partition_broadcast(P))
            return t

        binB = bcast("b_in")
        boB = bcast("bo")
        b2B = bcast("b2")
        boutB = bcast("b_out")
        bvB = bcast("bv", [P, H, HD])
        bqp = pc.tile([P, DK], F32, tag="bqp")
        nc.gpsimd.dma_start(out=bqp, in_=d["bqp"])
        b1c = pc.tile([P, FK], F32, tag="b1c")
        nc.gpsimd.dma_start(out=b1c, in_=d["b1c"])
        b1cs = pc.tile([P, FK], F32, tag="b1cs")
        nc.any.tensor_scalar(b1cs, b1c, scalar1=1.702, scalar2=None, op0=OP.mult)

    # ---- HAM warm-up: real matmuls (transposes don't count as PE-busy).
    # 512-wide streams from the memset tile span the DMA wait until xT/w_in
    # arrive (~12us), lifting the PE clock to K=8/8 before h1 starts.
    warm_ctr = [0]

    def warm(n):
        for _ in range(n):
            i = warm_ctr[0]
            warm_ctr[0] += 1
            wt = ps_mm(f"warm{i}")
            nc.tensor.matmul(wt, warmw[:, 0:128], warmw, start=True, stop=True)

    warm(20)

    # ---- h1 = x @ w_in (+ b_in) ----
    h1 = ph.tile([P, SM, D], F32, tag="h", name="h1")

    def emit_h1(m):
        ps = ps_mm(f"h1ps{m}")
        for dk in range(DK):
            nc.tensor.matmul(
                ps,
                xT_sb[:, m // 4, dk, (m % 4) * P : (m % 4 + 1) * P],
                win_sb[:, dk, :],
                start=(dk == 0),
                stop=(dk == DK - 1),
            )
        if zb:
            nc.vector.tensor_copy(h1[:, m, :], ps)
        else:
            nc.vector.tensor_tensor(out=h1[:, m, :], in0=ps, in1=binB, op=OP.add)

    # ---- batched LN: stats (DVE) + rs = exp(-0.5*ln(var+eps)) (ACT) ----
    def ln_stats(src_rows, mv, lo):
        # bn_stats/aggr for 4 rows into mv[:, lo:lo+4, :]
        for i, src in enumerate(src_rows):
            st = psm.tile([P, 6], F32, tag="st", name=f"st{lo + i}")
            nc.vector.bn_stats(st, src)
            nc.vector.bn_aggr(mv[:, lo + i, :], st)

    def ln_apply(src_rows, mv, lo, hn_tag, after=None):
        # (ln_apply.last_exp holds the most recent Exp inst for epoch pinning)
        n = len(src_rows)
        lnv = psm.tile([P, 4], F32, tag="lnv", name=f"lnv{lo}")
        ln_inst = nc.scalar.activation(
            lnv[:, 0:n], mv[:, lo : lo + n, 1], AF.Ln, bias=epsc
        )
        if after is not None:
            # keep the ACT queue in one table-set epoch: this Ln must not be
            # scheduled between Gelu ops (each crossing costs a ~2.7us
            # ACT_TABLE_LOAD)
            tile.add_dep_helper(ln_inst.ins, after.ins, reason="ln after gelu epoch")
        rs = psm.tile([P, 4], F32, tag="rs", name=f"rs{lo}")
        ln_apply.last_exp = nc.scalar.activation(
            rs[:, 0:n], lnv[:, 0:n], AF.Exp, scale=-0.5
        )
        ng = psm.tile([P, 4], F32, tag="ng", name=f"ng{lo}")
        nc.vector.tensor_tensor(
            out=ng[:, 0:n], in0=mv[:, lo : lo + n, 0], in1=rs[:, 0:n], op=OP.mult
        )
        hns = []
        for i, src in enumerate(src_rows):
            hn = phn.tile([P, D], BF, tag="hn", name=f"{hn_tag}{lo + i}")
            nc.vector.tensor_scalar(
                hn, src, scalar1=rs[:, i : i + 1], scalar2=ng[:, i : i + 1],
                op0=OP.mult, op1=OP.subtract,
            )
            hns.append(hn)
        return hns

    def transpose_row(hT, m, src):
        t4 = pps.tile([P, DK, P], BF, tag="aux", name=f"tr{m}")
        for dk in range(DK):
            nc.tensor.transpose(t4[:, dk, :], src[:, dk * P : (dk + 1) * P], identB)
        nc.vector.tensor_copy(hT[:, :, m * P : (m + 1) * P], t4)

    # ---- v projection into v_aug [t, m, h, hd+denom] ----
    hn1T = phT.tile([P, DK, S], BF, tag="hT", name="hn1T")
    v_aug = pva.tile([P, SM, H, HD + 1], BF, tag="vaug")
    nc.gpsimd.memset(v_aug[:, :, :, HD : HD + 1], 1.0)

    def emit_v(m):
        psv = ps_mm(f"psv{m}", [P, H, HD])
        for dk in range(DK):
            nc.tensor.matmul(
                psv,
                hn1T[:, dk, m * P : (m + 1) * P],
                wv_sb[:, dk, :],
                start=(dk == 0),
                stop=(dk == DK - 1),
            )
        if zb:
            nc.vector.tensor_copy(v_aug[:, m, :, 0:HD], psv)
        else:
            nc.vector.tensor_tensor(out=v_aug[:, m, :, 0:HD], in0=psv, in1=bvB, op=OP.add)

    mv1 = psm.tile([P, SM, 2], F32, tag="mv", name="mv1")
    for m in range(4):
        emit_h1(m)
    ln_stats([h1[:, m, :] for m in range(4)], mv1, 0)
    for m in range(4, SM):
        emit_h1(m)
    hnA = ln_apply([h1[:, m, :] for m in range(4)], mv1, 0, "hn1_")
    ln_stats([h1[:, m, :] for m in range(4, SM)], mv1, 4)
    for m in range(4):
        transpose_row(hn1T, m, hnA[m])
        emit_v(m)
    hnB = ln_apply([h1[:, m, :] for m in range(4, SM)], mv1, 4, "hn1_")
    for m in range(4, SM):
        transpose_row(hn1T, m, hnB[m - 4])
    # v rows 4-7 are emitted inside pipeline step 1 as PE filler while the
    # first pair's exps drain on ACT; qk(pair) heads the pipeline below

    # ---- attention ----
    attn_nat = pattn.tile([P, SM, D], BF, tag="attn")
    qk_t = {}
    exp_t = {}

    def alloc_qk(p):
        ha, hb = 2 * p, 2 * p + 1
        for h in (ha, hb):
            qTa = pqk.tile([HD + AUG, S], BF, tag="qTa", name=f"qTa{h}")
            kTa = pqk.tile([HD + AUG, S], BF, tag="kTa", name=f"kTa{h}")
            nc.gpsimd.dma_start(
                out=qTa[HD : HD + AUG, :], in_=d["qaug"][3 * h : 3 * h + AUG, :]
            )
            nc.gpsimd.dma_start(
                out=kTa[HD : HD + AUG, :], in_=d["kaug"][3 * h : 3 * h + AUG, :]
            )
            qk_t[h] = (qTa, kTa)

    def emit_qk_part(p, is_q):
        ha, hb = 2 * p, 2 * p + 1
        w_sb = wq_sb if is_q else wk_sb
        qp = pqp.tile([P, S], BF, tag="qp", name=f"qp{p}{int(is_q)}")
        for half in range(2):
            psq = ps_mm(f"psq{p}{int(is_q)}{half}")
            for dk in range(DK):
                nc.tensor.matmul(
                    psq,
                    w_sb[:, dk, p * P : (p + 1) * P],
                    hn1T[:, dk, half * 512 : (half + 1) * 512],
                    start=(dk == 0),
                    stop=(dk == DK - 1),
                )
            dst = qp[:, half * 512 : (half + 1) * 512]
            if is_q and not zb:
                nc.vector.tensor_scalar(
                    dst, psq, scalar1=bqp[:, p : p + 1], scalar2=None, op0=OP.add
                )
            else:
                nc.vector.tensor_copy(dst, psq)
        # partition-shift the two heads' halves into the aug tiles
        idx = 0 if is_q else 1
        nc.sync.dma_start(out=qk_t[ha][idx][0:HD, :], in_=qp[0:HD, :])
        nc.sync.dma_start(out=qk_t[hb][idx][0:HD, :], in_=qp[HD:P, :])

    def emit_scores_tile(h, ti):
        qTa, kTa = qk_t[h]
        expT = exp_t[h]
        t0, tw, tp = _score_tiles(h)[ti]
        sc = ps_sc(f"sc{h}_{ti}")
        for (j, ss, do, w) in tp:
            nc.tensor.matmul(
                sc[:, do : do + w],
                kTa[:, j * P : (j + 1) * P],
                qTa[:, ss : ss + w],
                start=True,
                stop=True,
            )
        nc.scalar.activation(expT[:, t0 : t0 + tw], sc[:, 0:tw], AF.Exp, scale=0.125)
        # diagonal masks living in this tile (DVE: gpsimd masks contend
        # with DVE's shared SBUF port and slow every DVE op ~20%)
        msk = maskf if _fwd(h) else maskb
        for j in range(SM):
            dg = _diag_off(h, j)
            if t0 <= dg < t0 + tw:
                nc.vector.tensor_tensor(
                    out=expT[:, dg : dg + P], in0=expT[:, dg : dg + P], in1=msk,
                    op=OP.mult,
                )

    def alloc_expT(h):
        small = _band(h) <= 2
        pool_ = pexpS if small else pexpL
        tag = "expS" if small else "expL"
        width = EW_S if small else EW_L
        exp_t[h] = pool_.tile([P, width], BF, tag=tag, name=f"expT{h}")

    def emit_pv_group(h, mg):
        expT = exp_t[h]
        pvt = pps.tile([P, 4, HD + 1], F32, tag="aux", name=f"pv{h}_{mg}")
        for mi in range(4):
            m = mg * 4 + mi
            js = _pv_js(h, m)
            for i, j in enumerate(js):
                col = _pv_col(h, j, m)
                nc.tensor.matmul(
                    pvt[:, mi, :],
                    expT[:, col : col + P],
                    v_aug[:, j, h, :],
                    start=(i == 0),
                    stop=(i == len(js) - 1),
                )
        rinv = psm.tile([P, 4], F32, tag="rinv", name=f"rinv{h}_{mg}")
        nc.vector.reciprocal(rinv, pvt[:, :, HD])
        for mi in range(4):
            m = mg * 4 + mi
            nc.vector.tensor_scalar(
                attn_nat[:, m, h * HD : (h + 1) * HD],
                pvt[:, mi, 0:HD],
                scalar1=rinv[:, mi : mi + 1],
                scalar2=None,
                op0=OP.mult,
            )

    # wo-phase tiles/helpers (emit_wo_h2 is interleaved into pipeline
    # step 5: the pv-only tail is all thin N=65 matmuls, which trips the
    # HAM thin-M throttle unless wide wo matmuls are woven between)
    attnT2 = phT.tile([P, DK, S], BF, tag="hT", name="attnT2")
    h2 = ph.tile([P, SM, D], F32, tag="h", name="h2")

    def emit_wo_h2(m):
        transpose_row(attnT2, m, attn_nat[:, m, :])
        ps = ps_mm(f"pswo{m}")
        for dk in range(DK):
            nc.tensor.matmul(
                ps,
                attnT2[:, dk, m * P : (m + 1) * P],
                wo_sb[:, dk, :],
                start=(dk == 0),
                stop=(dk == DK - 1),
            )
        nc.vector.tensor_tensor(out=h2[:, m, :], in0=ps, in1=h1[:, m, :], op=OP.add)
        if not zb:
            nc.gpsimd.tensor_tensor(out=h2[:, m, :], in0=h2[:, m, :], in1=boB, op=OP.add)

    # software pipeline over head pairs: qk(p) | scores(p-1) | pv(p-2).
    # Within a step the score-tile fills are emitted FIRST (the ACT exp
    # chain is the step's pacer; it must start immediately), with qk and
    # pv matmul work woven between fills to keep PE fed while the sc psum
    # rotation drains through ACT.
    # pair processing order: big-band pairs (2,3) and (6,7) first, so the
    # pipeline's final ACT-exp drain belongs to a small-band pair (short)
    PAIRS = [1, 3, 0, 2]
    for step in range(6):
        if step < 4:
            alloc_qk(PAIRS[step])
        scw, other = [], []
        if 1 <= step <= 4:
            pp_ = PAIRS[step - 1]
            ha, hb = 2 * pp_, 2 * pp_ + 1
            alloc_expT(ha)
            alloc_expT(hb)
            for h in (ha, hb):
                for ti in range(len(_score_tiles(h))):
                    scw.append(("sc", h, ti))
        if step < 4:
            other.append(("qk", PAIRS[step], True))
            other.append(("qk", PAIRS[step], False))
        if step == 1:
            for m in range(4, SM):  # deferred v rows: PE filler
                other.append(("v", m, 0))
        if step >= 2:
            # mg-major order: both heads' m0-3 groups first, so the wo
            # phase (which consumes attn_nat row-major) can start early
            pp_ = PAIRS[step - 2]
            for mg in range(2):
                for h in (2 * pp_, 2 * pp_ + 1):
                    other.append(("pv", h, mg))
        if step == 5:
            # weave the first wo rows between the thin pv groups
            other = [other[0], other[1], ("wo", 0, 0), other[2], ("wo", 1, 0),
                     other[3], ("wo", 2, 0), ("wo", 3, 0)]
        out = []
        si, oi = 0, 0
        # 2-tile ACT head start, then alternate
        while si < len(scw) or oi < len(other):
            for _ in range(2 if si == 0 else 1):
                if si < len(scw):
                    out.append(scw[si]); si += 1
            if oi < len(other):
                out.append(other[oi]); oi += 1
        for kind, a, b in out:
            if kind == "sc":
                emit_scores_tile(a, b)
            elif kind == "qk":
                emit_qk_part(a, b)
            elif kind == "wo":
                emit_wo_h2(a)
            elif kind == "v":
                emit_v(a)
            else:
                emit_pv_group(a, b)
        if step >= 2:
            pp_ = PAIRS[step - 2]
            for h in (2 * pp_, 2 * pp_ + 1):
                exp_t.pop(h)
                qk_t.pop(h)

    # LN2 in 2-row batches pipelined into the wo stream, so each short
    # stats->ln->exp->hn chain hides under the next two wo matmul groups
    mv2 = psm.tile([P, SM, 2], F32, tag="mv", name="mv2")
    hn2T = phT.tile([P, DK, S], BF, tag="hT", name="hn2T")
    hn2 = [None] * SM

    def ln2_batch(lo):
        ln_stats([h2[:, m, :] for m in (lo, lo + 1)], mv2, lo)
        hns = ln_apply([h2[:, m, :] for m in (lo, lo + 1)], mv2, lo, "hn2_")
        hn2[lo], hn2[lo + 1] = hns

    ln2_batch(0)
    emit_wo_h2(4)
    ln2_batch(2)
    emit_wo_h2(5)
    transpose_row(hn2T, 0, hn2[0])
    transpose_row(hn2T, 1, hn2[1])
    emit_wo_h2(6)
    ln2_batch(4)
    transpose_row(hn2T, 2, hn2[2])
    transpose_row(hn2T, 3, hn2[3])
    emit_wo_h2(7)
    ln2_batch(6)
    # hn2T rows 4-7 are transposed INSIDE the FFN stream below: ffn1-half0
    # only needs rows 0-3, so the PE need not park behind the LN2-B chain

    # release attention-phase SBUF (LIFO) before allocating the FFN gt stage
    pexpL.release()
    pexpS.release()
    pqp.release()
    pqk.release()

    pg = tc.alloc_tile_pool(name="gelu", bufs=2)

    # ---- FFN: per half, ffn1+gelu into gt, then ffn2 per s-chunk ----
    h3 = ph.tile([P, SM, D], F32, tag="h", name="h3")

    def emit_ffn1_group(half, g, gt):
        # kc pair (2g, 2g+1) -> one 2-bank psum tile -> one gelu
        sc = ps_sc(f"f1_{half}_{g}", [P, 2, 512])
        for i in range(2):
            kc = 2 * g + i
            for dk in range(DK):
                nc.tensor.matmul(
                    sc[:, i, :],
                    w1_sb[:, dk, kc * P : (kc + 1) * P],
                    hn2T[:, dk, half * 512 : (half + 1) * 512],
                    start=(dk == 0),
                    stop=(dk == DK - 1),
                )
        if gelu_mode == "gelu":
            if zb:
                return nc.scalar.activation(gt[:, 2 * g : 2 * g + 2, :], sc, AF.Gelu)
            last = None
            for i in range(2):
                kc = 2 * g + i
                last = nc.scalar.activation(
                    gt[:, kc, :], sc[:, i, :], AF.Gelu, bias=b1c[:, kc : kc + 1]
                )
            return last
        # CoreSim lacks Gelu: x*sigmoid(1.702x) stand-in
        last = None
        for i in range(2):
            kc = 2 * g + i
            scs = sc[:, i, :]
            sg = pg.tile([P, 512], F32, tag="sg")
            if zb:
                last = nc.scalar.activation(sg, scs, AF.Sigmoid, scale=1.702)
                nc.vector.tensor_tensor(out=gt[:, kc, :], in0=sg, in1=scs, op=OP.mult)
            else:
                last = nc.scalar.activation(
                    sg, scs, AF.Sigmoid, bias=b1cs[:, kc : kc + 1], scale=1.702
                )
                xb = pg.tile([P, 512], F32, tag="xb")
                nc.any.tensor_scalar(
                    xb, scs, scalar1=b1c[:, kc : kc + 1], scalar2=None, op0=OP.add
                )
                nc.any.tensor_tensor(out=gt[:, kc, :], in0=sg, in1=xb, op=OP.mult)
        return last

    def emit_ffn2_m(half, mm, gt):
        acc = ps_mm(f"f2_{half}_{mm}")
        for kc in range(FK):
            nc.tensor.matmul(
                acc,
                gt[:, kc, mm * P : (mm + 1) * P],
                w2_sb[:, kc, :],
                start=(kc == 0),
                stop=(kc == FK - 1),
            )
        m = half * 4 + mm
        nc.vector.tensor_tensor(out=h3[:, m, :], in0=acc, in1=h2[:, m, :], op=OP.add)
        if not zb:
            nc.gpsimd.tensor_tensor(out=h3[:, m, :], in0=h3[:, m, :], in1=b2B, op=OP.add)

    gts = {}
    for half in range(2):
        gts[half] = pg.tile([P, FK, 512], BF, tag="gt", name=f"gt{half}")
    # chain the gelu ops after LN2-B's Exp and after each other so the
    # scheduler cannot interleave them with ln/exp ops (ACT table thrash)
    last_gelu = ln_apply.last_exp

    def chain_ffn1(half, g, gt):
        nonlocal last_gelu
        prev, last_gelu = last_gelu, emit_ffn1_group(half, g, gt)
        tile.add_dep_helper(last_gelu.ins, prev.ins, reason="gelu epoch chain")

    # order: all 16 ffn1 groups first (deferred LN2-B transposes woven into
    # the early stream), THEN ffn2 half 0, THEN ffn2 half 1.  ffn2(0) is
    # ready work that covers the ACT drain of half-1's last gelus, so
    # ffn2(1) never waits on gelu(1,7).
    for g in range(2):
        chain_ffn1(0, g, gts[0])
    for m in (4, 5):  # deferred LN2-B transposes ride the FFN stream
        transpose_row(hn2T, m, hn2[m])
    for g in range(2, 4):
        chain_ffn1(0, g, gts[0])
    for m in (6, 7):
        transpose_row(hn2T, m, hn2[m])
    for g in range(4, 8):
        chain_ffn1(0, g, gts[0])
    for g in range(8):
        chain_ffn1(1, g, gts[1])
    mvf = psm.tile([P, SM, 2], F32, tag="mv", name="mvf")
    for mm in range(4):
        emit_ffn2_m(0, mm, gts[0])
        ln_stats([h3[:, mm, :]], mvf, mm)
    # LNf-A emitted BEFORE the half-1 ffn2 chain: its DVE hn-writes sit
    # ahead of the h3-adds in the in-order DVE queue and complete while
    # the PE is still in ffn2, so the w_out tail starts with zero stall.
    hnfA = ln_apply([h3[:, m, :] for m in range(4)], mvf, 0, "hn3_", after=last_gelu)
    for mm in range(4):
        emit_ffn2_m(1, mm, gts[1])
        ln_stats([h3[:, 4 + mm, :]], mvf, 4 + mm)

    # ---- LNf + w_out tail ----
    hn3T = phT.tile([P, DK, S], BF, tag="hT", name="hn3T")
    out_view = d["out"].rearrange("(c p) n -> p c n", p=P)

    def emit_wout(m):
        ps = ps_mm(f"psout{m}")
        for dk in range(DK):
            nc.tensor.matmul(
                ps,
                hn3T[:, dk, m * P : (m + 1) * P],
                wout_sb[:, dk, :],
                start=(dk == 0),
                stop=(dk == DK - 1),
            )
        osb = posb.tile([P, D], F32, tag="osb")
        if zb and m % 2 == 0:
            # ACT is idle at the tail; keeping half of these off DVE lets
            # the hn3T transpose-copies (which gate wout) flow without
            # queuing, and splits the final drain across two engines
            nc.scalar.activation(osb, ps, AF.Copy)
        elif zb:
            nc.vector.tensor_copy(osb, ps)
        else:
            nc.vector.tensor_tensor(out=osb, in0=ps, in1=boutB, op=OP.add)
        nc.sync.dma_start(out=out_view[:, m, :], in_=osb)

    for m in range(4):
        transpose_row(hn3T, m, hnfA[m])
        emit_wout(m)
        if m == 3:
            # LNf-B chains on DVE/ACT while the PE runs the first wout rows
            hnfB = ln_apply([h3[:, mm, :] for mm in range(4, SM)], mvf, 4, "hn3_")
    for m in range(4, SM):
        transpose_row(hn3T, m, hnfB[m - 4])
        emit_wout(m)

    for p_ in (pg, posb, pattn, pva, psm, phn, phT, ph, pw, pc, pps):
        p_.release()


def host_prep(inputs):
    """Fold LN affine params into weights; build ALiBi helper tensors."""
    f = lambda k: np.asarray(inputs[k], dtype=np.float64)
    ln1_s, ln1_b = f("ln1_s"), f("ln1_b")
    ln2_s, ln2_b = f("ln2_s"), f("ln2_b")
    lnf_s, lnf_b = f("lnf_s"), f("lnf_b")
    wq, bq = f("wq"), f("bq")
    wk = f("wk")
    wv, bv = f("wv"), f("bv")
    w1, b1 = f("w1"), f("b1")
    w_out, b_out = f("w_out"), f("b_out")

    wq_f = ln1_s[:, None] * wq
    bq_f = (bq + ln1_b @ wq).astype(np.float32)
    wk_f = ln1_s[:, None] * wk
    wv_f = ln1_s[:, None] * wv
    bv_f = (bv + ln1_b @ wv).astype(np.float32)
    w1_f = ln2_s[:, None] * w1
    b1_f = (b1 + ln2_b @ w1).astype(np.float32)
    wout_f = lnf_s[:, None] * w_out
    bout_f = (b_out + lnf_b @ w_out).astype(np.float32)

    sl = _slopes()
    qaug = np.zeros((H, 3, S), np.float64)
    kaug = np.zeros((H, 3, S), np.float64)
    s_idx = np.arange(S, dtype=np.float64)
    for h in range(H):
        sgn = -1.0 if h < H // 2 else 1.0  # sign of the per-s row term
        kraw = -sgn * 8.0 * sl[h % 4] * s_idx  # per-t term, rides kTa aug rows
        hi = kraw.astype(BF_NP).astype(np.float64)
        kaug[h, 0] = hi
        kaug[h, 1] = kraw - hi  # bf16 residual: per-t term exact to ~2^-18
        kaug[h, 2] = 1.0
        qaug[h, 0] = 1.0
        qaug[h, 1] = 1.0
        qaug[h, 2] = sgn * 8.0 * sl[h % 4] * s_idx  # per-s term (cancels)
    qaug = qaug.reshape(3 * H, S)
    kaug = kaug.reshape(3 * H, S)
    maskf = np.triu(np.ones((P, P), np.float32))  # keep t <= s (p <= c)
    maskb = np.tril(np.ones((P, P), np.float32))  # keep t >= s (p >= c)

    bf = lambda a: np.ascontiguousarray(np.asarray(a, np.float32).astype(BF_NP))

    def chunked(w):
        # [K, N] -> on-chip [p, c, n] layout with K = c*128 + p
        w = np.asarray(w, np.float32)
        k, n = w.shape
        return bf(w.reshape(k // P, P, n).transpose(1, 0, 2))

    zero_bias = all(
        float(np.abs(a).max()) == 0.0
        for a in (f("b_in"), bq_f, bv_f, f("bo"), b1_f, f("b2"), bout_f)
    )
    common = {
        "w_in": chunked(inputs["w_in"]),
        "wq": chunked(wq_f),
        "wk": chunked(wk_f),
        "wv": chunked(wv_f),
        "wo": chunked(inputs["wo"]),
        "w1": chunked(w1_f),
        "w2": chunked(inputs["w2"]),
        "w_out": chunked(wout_f),
        "qaug": bf(qaug),
        "kaug": bf(kaug),
        "maskf": bf(maskf),
        "maskb": bf(maskb),
        "ident": bf(np.eye(P, dtype=np.float32)),
    }
    if not zero_bias:
        common.update(
            {
                "b_in": np.asarray(inputs["b_in"], np.float32),
                "bo": np.asarray(inputs["bo"], np.float32),
                "b2": np.asarray(inputs["b2"], np.float32),
                "b_out": bout_f,
                "bv": bv_f,
                # paired q bias: [128 dims of the pair, pair index]
                "bqp": np.ascontiguousarray(bq_f.reshape(DK, P).T),
                "b1c": np.ascontiguousarray(b1_f.reshape(FK, P).T),
            }
        )
    return common, zero_bias


def core_map(common, x, i):
    xT = np.asarray(x[i], np.float32).T  # [D, S]
    xT = xT.reshape(DK, P, S).transpose(1, 0, 2).astype(BF_NP)
    return dict(
        common,
        xT0=np.ascontiguousarray(xT[:, :, 0:512]),
        xT1=np.ascontiguousarray(xT[:, :, 512:1024]),
    )


_NC_CACHE = {}


def get_nc(gelu_mode="gelu", zero_bias=True):
    key = (gelu_mode, zero_bias)
    if key not in _NC_CACHE:
        _NC_CACHE[key] = build_nc(gelu_mode, zero_bias)
    return _NC_CACHE[key]


def run(inputs, trace=False, tmpdir=None):
    common, zero_bias = host_prep(inputs)
    x = np.asarray(inputs["x"], np.float32)
    in_maps = [core_map(common, x, i) for i in range(N_CORES)]
    nc = get_nc("gelu", zero_bias)
    res = run_bass_kernel_spmd(
        nc, in_maps, core_ids=list(range(N_CORES)), trace=trace, tmpdir=tmpdir
    )
    out = np.stack([res.results[i]["out"] for i in range(N_CORES)])
    return out.astype(np.float32), res


def kernel(**inputs):
    out, _ = run(inputs, trace=False)
    return out
